# revision 40
# baseline (speedup 1.0000x reference)
"""Trainium2 Bass kernel for MemexQA-FVTA (dense transformer block), v3.

Data-parallel over batch across 8 NeuronCores (8 elems/core, no collectives).

v3 over v2: 32-granular per-slot token geometry. Each of the 8 per-core
slots bakes (TB, IB) = (text tokens padded to 32, image tokens padded to
32), slot-max over the 8 cores; elements are assigned to slots by sorting
on text length, splitting by image length, then local swaps minimizing
sum(TB+IB). All attention/FVTA matmuls use exact free dims (Ne = TB+IB)
and partial-partition stationary tiles, cutting padded-token MACs ~17%
vs the 128-granular v2 tiling.
"""

import sys
import numpy as np

H, B, LT, LI, LQ = 2, 64, 384, 128, 24
D, KD, VD = 768, 384, 384
DV = H * VD          # 768
NCORES = 8
BL = B // NCORES     # 8 batch elements per core
NKC = D // 128       # 6 contraction chunks
MASK = -30000.0
EPS = 1e-5
MAGIC = 0x5F3759DF


def _ensure_path():
    try:
        import concourse  # noqa: F401
    except ImportError:
        sys.path.insert(0, "/opt/trn_rl_repo")


_COMPILED = {}


def _geom(slots):
    """Derived per-slot geometry. slots: tuple of (TB, IB)."""
    TP = []    # text tile partition sizes
    KT = []    # key tile sizes (text tiles + img tile)
    KS = []    # key tile token-axis start columns
    NE = []    # concat token count
    CQ = []    # concat-axis 128-tile sizes
    for TB, IB in slots:
        tp = [min(128, TB - 128 * t) for t in range((TB + 127) // 128)]
        TP.append(tp)
        KT.append(tp + [IB])
        KS.append([128 * t for t in range(len(tp))] + [TB])
        ne = TB + IB
        NE.append(ne)
        CQ.append([min(128, ne - 128 * c) for c in range((ne + 127) // 128)])
    KOFF = [0]
    for k in KT:
        KOFF.append(KOFF[-1] + len(k))
    COFF = [0]
    for c in CQ:
        COFF.append(COFF[-1] + len(c))
    return TP, KT, KS, NE, CQ, KOFF, COFF


def build_nc(slots):
    """Build + compile the per-core Bass program for the given per-slot
    (TB, IB) pairs (tuple of 8 pairs, 32-granular). Cached."""
    global _COMPILED
    slots = tuple((int(a), int(b)) for a, b in slots)
    if slots in _COMPILED:
        return _COMPILED[slots]
    _ensure_path()
    from contextlib import ExitStack

    import concourse.bacc as bacc
    import concourse.tile as tile
    import concourse.mybir as mybir

    f32 = mybir.dt.float32
    f32r = mybir.dt.float32r
    bf16 = mybir.dt.bfloat16
    f8 = mybir.dt.float8e4
    i32 = mybir.dt.int32
    AF = mybir.ActivationFunctionType
    ALU = mybir.AluOpType
    AX = mybir.AxisListType

    TP, KT, KS, NE, CQ, KOFF, COFF = _geom(slots)
    TOTK = KOFF[-1]
    TOTC = COFF[-1]

    nc = bacc.Bacc("TRN2", target_bir_lowering=False, debug=False,
                   num_devices=NCORES)

    def din(name, shape, dt=None):
        return nc.declare_dram_parameter(name, list(shape), dt or f32r, False).ap()

    text_d = din("text", [BL, LT, D])
    images_d = din("images", [BL, LI, D])
    query_d = din("query", [BL, LQ, D], bf16)
    # packed proj weights: [H, 128, NKC*384]; [h, p, kc*384+m] = W[h, kc*128+p, m]
    w_d = {n: din(n, [H, 128, NKC * 384])
           for n in ["wsq", "wiq", "wsk", "wik", "wsv", "wiv"]}
    wkp_d = din("wkp", [128, 36 * 128], bf16)   # [p,(vdc*6+dkc)*128+m]
    wvp_d = din("wvp", [128, 12 * 384], bf16)   # [p,(vdc*2+hf)*384+m]
    bqk_d = din("bqk", [128, 24], f32)          # col=(qk*2+h)*3+mf (+12 img)
    bkp_d = din("bkp_t", [128, 6], f32)
    bv_si_d = din("bv_si", [1, 4 * VD])         # seg = si*2 + h
    kmT_d = din("kmT", [128, TOTK], f32)        # per-partition key-tile mask
    MOFF = [0]
    for ne in NE:
        MOFF.append(MOFF[-1] + ne)
    maskr_d = din("maskr", [1, MOFF[-1]], f8)   # FVTA token mask row (0/-240)
    qvalid2_d = din("qvalid2", [LQ, BL], f32)   # [q, j]: (q < ql)/LQ
    ident_d = din("ident_r", [128, 128])
    ones_d = din("ones_r", [1, 128])
    out_d = nc.declare_dram_parameter("out", [BL, DV], f32, True).ap()

    with tile.TileContext(nc) as tc, ExitStack() as ctx:
        def pool(**kw):
            return ctx.enter_context(tc.tile_pool(**kw))

        cpool = pool(name="const", bufs=1)
        wpool = pool(name="wres", bufs=1)
        lnp = pool(name="ln", bufs=2)
        scrq = pool(name="scrq", bufs=1)
        stat = pool(name="stat", bufs=3)
        xtp = pool(name="xt", bufs=2)
        qkp = pool(name="qk", bufs=1)
        vp = pool(name="v", bufs=1)
        etp = pool(name="et", bufs=1)
        sinvp = pool(name="sinv", bufs=1)
        albp = pool(name="alb", bufs=1)
        valsp = pool(name="vals", bufs=1)
        keysp = pool(name="keys", bufs=1)
        qftp = pool(name="qft", bufs=2)
        smallp = pool(name="sml", bufs=1)
        pmm = pool(name="pmm", bufs=5, space="PSUM")
        ptr = pool(name="ptr", bufs=2, space="PSUM")
        psml = pool(name="psml", bufs=1, space="PSUM")

        def r(ap):
            return ap.bitcast(f32r)

        def p(ap):
            return ap.bitcast(f32)

        # ---- constants ----
        ident = cpool.tile([128, 128], f32r, tag="ident")
        nc.sync.dma_start(ident[:], ident_d[:])
        ones1 = cpool.tile([1, 128], f32r, tag="ones1")
        nc.sync.dma_start(ones1[:], ones_d[:])
        negI = cpool.tile([128, 128], f32, tag="negI")
        nc.gpsimd.memset(negI[:], 0.0)
        nc.gpsimd.affine_select(
            out=negI[:], in_=negI[:], compare_op=ALU.not_equal, fill=MASK,
            base=0, pattern=[[-1, 128]], channel_multiplier=1)
        ones_bf = cpool.tile([128, 1], bf16, tag="onesbf")
        nc.gpsimd.memset(ones_bf[:], 1.0)
        bqk = cpool.tile([128, 24], f32, tag="bqk")
        nc.sync.dma_start(bqk[:], bqk_d[:])
        bkp = cpool.tile([128, 6], f32, tag="bkp")
        nc.sync.dma_start(bkp[:], bkp_d[:])

        kmT = cpool.tile([128, TOTK], f32, tag="kmT")
        nc.sync.dma_start(kmT[:], kmT_d[:])
        maskr = cpool.tile([1, MOFF[-1]], f8, tag="maskr")
        nc.sync.dma_start(maskr[:], maskr_d[:])
        onesr_f8 = cpool.tile([1, LQ], f8, tag="onesrf8")
        nc.gpsimd.memset(onesr_f8[:], 1.0)
        identbf = cpool.tile([128, 128], bf16, tag="identbf")
        nc.gpsimd.memset(identbf[:], 0.0)
        nc.gpsimd.affine_select(
            out=identbf[:], in_=identbf[:], compare_op=ALU.not_equal,
            fill=1.0, base=0, pattern=[[-1, 128]], channel_multiplier=1)

        qvalid2 = cpool.tile([LQ, BL], f32, tag="qvalid2")
        nc.sync.dma_start(qvalid2[:], qvalid2_d[:])

        wt = {}

        # ================= per-elem pipeline =================
        def preamble(j):
            """DMA + LN + transposes for slot j. Returns (qfT, xT)."""
            TB, IB = slots[j]
            Ne = NE[j]
            qtile = scrq.tile([LQ, D], bf16, tag="q")
            nc.scalar.dma_start(qtile[:], query_d[j])
            qfT = qftp.tile([128, NKC * LQ], bf16, tag="qfT")
            for half, nkc in ((0, 4), (1, 2)):
                pt = ptr.tile([128, 512], f32r, tag="tr")
                pt_bf = pt.bitcast(bf16)
                for i in range(nkc):
                    kc = half * 4 + i
                    nc.tensor.transpose(pt_bf[:, i * LQ:(i + 1) * LQ],
                                        qtile[:, kc * 128:(kc + 1) * 128],
                                        identbf[0:LQ, 0:LQ])
                nc.vector.tensor_copy(
                    qfT[:, half * 4 * LQ: half * 4 * LQ + nkc * LQ],
                    pt_bf[:, 0:nkc * LQ])
            xT = xtp.tile([128, NKC * 512], f32r, tag="xT")
            xT3 = xT[:, 0:NKC * Ne].rearrange("p (k n) -> p k n", k=NKC)
            for tt, (P, g0) in enumerate(zip(KT[j], KS[j])):
                if tt < len(TP[j]):
                    src = text_d[j, 128 * tt: 128 * tt + P, :]
                else:
                    src = images_d[j, 0:P, :]
                x = lnp.tile([128, D], f32r, tag="x")
                nc.scalar.dma_start(x[0:P, :], src)
                bnst = stat.tile([128, 12], f32, tag="bnst")
                nc.vector.bn_stats(bnst[0:P, 0:6], p(x[0:P, 0:384]))
                nc.vector.bn_stats(bnst[0:P, 6:12], p(x[0:P, 384:768]))
                mv = stat.tile([128, 2], f32, tag="mv")
                nc.vector.bn_aggr(mv[0:P, :], bnst[0:P, :])
                mu = mv[0:P, 0:1]
                ve = stat.tile([128, 1], f32, tag="ve")
                nc.gpsimd.tensor_scalar_add(ve[0:P, :], mv[0:P, 1:2], EPS)
                # Newton rsqrt: seed from bitcast magic, 2 iterations
                t1 = stat.tile([128, 1], i32, tag="t1")
                nc.vector.tensor_scalar(t1[0:P, :], ve[0:P, :].bitcast(i32),
                                        1, None, ALU.logical_shift_right)
                t2 = stat.tile([128, 1], i32, tag="t2")
                nc.vector.tensor_scalar_mul(t2[0:P, :], t1[0:P, :], -1)
                nc.vector.tensor_scalar_add(t2[0:P, :], t2[0:P, :], MAGIC)
                y0 = t2[0:P, :].bitcast(f32)
                hh = stat.tile([128, 1], f32, tag="hh")
                nc.vector.tensor_scalar_mul(hh[0:P, :], ve[0:P, :], -0.5)
                z = stat.tile([128, 1], f32, tag="z")
                y1 = stat.tile([128, 1], f32, tag="y1")
                nc.vector.tensor_mul(z[0:P, :], y0, y0)
                nc.vector.tensor_mul(z[0:P, :], z[0:P, :], hh[0:P, :])
                nc.vector.tensor_scalar_add(z[0:P, :], z[0:P, :], 1.5)
                nc.vector.tensor_mul(y1[0:P, :], y0, z[0:P, :])
                rstd = stat.tile([128, 1], f32, tag="rstd")
                nc.vector.tensor_mul(z[0:P, :], y1[0:P, :], y1[0:P, :])
                nc.vector.tensor_mul(z[0:P, :], z[0:P, :], hh[0:P, :])
                nc.vector.tensor_scalar_add(z[0:P, :], z[0:P, :], 1.5)
                nc.vector.tensor_mul(rstd[0:P, :], y1[0:P, :], z[0:P, :])
                nma = stat.tile([128, 1], f32, tag="nma")
                nc.gpsimd.tensor_mul(nma[0:P, :], mu, rstd[0:P, :])
                nmr = stat.tile([128, 1], f32, tag="nmr")
                nc.gpsimd.tensor_scalar_mul(nmr[0:P, :], nma[0:P, :], -1.0)
                nc.scalar.activation(x[0:P, :], p(x[0:P, :]), AF.Identity,
                                     bias=nmr[0:P, 0:1], scale=rstd[0:P, 0:1])
                for half, nkc in ((0, 4), (1, 2)):
                    pt = ptr.tile([128, 512], f32r, tag="tr")
                    for i in range(nkc):
                        kc = half * 4 + i
                        nc.tensor.transpose(
                            r(pt[:, i * P:(i + 1) * P]),
                            r(x[0:P, kc * 128:(kc + 1) * 128]),
                            r(ident[0:P, 0:P]))
                    src3 = pt[:, 0:nkc * P].rearrange("p (k n) -> p k n",
                                                      k=nkc)
                    nc.scalar.activation(
                        xT3[:, half * 4: half * 4 + nkc, g0: g0 + P],
                        p(src3), AF.Identity)
            return qfT, xT

        def compute_heads(j, state):
            """q/k/v projections + transposed-softmax attention for slot j.
            Returns bf16 albumT [128, 6*Ne] (head-concat, normalized)."""
            qfT, xT = state
            TB, IB = slots[j]
            Ne = NE[j]
            tiles, starts = KT[j], KS[j]
            nkt = len(tiles)
            albumT = albp.tile([128, NKC * 512], bf16, tag="albumT")
            for h in range(H):
                qT = qkp.tile([128, 3 * 512], f32r, tag="qT")
                kT = qkp.tile([128, 3 * 512], f32r, tag="kT")
                for qk, dstT in ((0, qT), (1, kT)):
                    wS = wt[("wsq" if qk == 0 else "wsk", h)]
                    wI = wt[("wiq" if qk == 0 else "wik", h)]
                    if TB >= 160:
                        # text tokens: feature-major direct (full-rate TB)
                        for mf in range(3):
                            ps = pmm.tile([128, 512], f32, tag="mm")
                            for kc in range(NKC):
                                nc.tensor.matmul(
                                    ps[:, 0:TB],
                                    r(wS[:, kc * 384 + mf * 128:
                                         kc * 384 + mf * 128 + 128]),
                                    r(xT[:, kc * Ne: kc * Ne + TB]),
                                    start=(kc == 0), stop=(kc == NKC - 1))
                            bcol = (qk * 2 + h) * 3 + mf
                            nc.scalar.activation(
                                dstT[:, mf * Ne: mf * Ne + TB],
                                ps[:, 0:TB], AF.Identity,
                                bias=bqk[:, bcol:bcol + 1])
                        text_tcs = []
                    else:
                        # small text: token-major + transpose
                        text_tcs = list(range(nkt - 1))
                    for tc in text_tcs + [nkt - 1]:
                        istext = tc < nkt - 1
                        P, g0 = tiles[tc], starts[tc]
                        wX = wS if istext else wI
                        ps2 = pmm.tile([128, 512], f32, tag="mm")
                        for kc in range(NKC):
                            nc.tensor.matmul(
                                ps2[0:P, 0:KD],
                                r(xT[:, kc * Ne + g0: kc * Ne + g0 + P]),
                                r(wX[:, kc * 384:(kc + 1) * 384]),
                                start=(kc == 0), stop=(kc == NKC - 1))
                        tm = smallp.tile([128, KD], f32r, tag="e2")
                        nc.vector.tensor_copy(tm[0:P, :], ps2[0:P, 0:KD])
                        pt = ptr.tile([128, 512], f32r, tag="tr")
                        for mf in range(3):
                            nc.tensor.transpose(
                                r(pt[:, mf * P:(mf + 1) * P]),
                                r(tm[0:P, mf * 128:(mf + 1) * 128]),
                                r(ident[0:P, 0:P]))
                        for mf in range(3):
                            bcol = (0 if istext else 12) + (qk * 2 + h) * 3 + mf
                            nc.scalar.activation(
                                dstT[:, mf * Ne + g0: mf * Ne + g0 + P],
                                p(pt[:, mf * P:(mf + 1) * P]),
                                AF.Identity, bias=bqk[:, bcol:bcol + 1])
                # v: token-major; bias added via const broadcast tile on
                # the PSUM->SBUF copy
                v = vp.tile([128, 4 * VD], bf16, tag="v")
                for tc in range(nkt):
                    istext = tc < nkt - 1
                    P, g0 = tiles[tc], starts[tc]
                    wV = wt[("wsv" if istext else "wiv", h)]
                    ps = pmm.tile([128, 512], f32, tag="mm")
                    for kc in range(NKC):
                        nc.tensor.matmul(
                            ps[0:P, 0:VD],
                            r(xT[:, kc * Ne + g0: kc * Ne + g0 + P]),
                            r(wV[:, kc * 384:(kc + 1) * 384]),
                            start=(kc == 0), stop=(kc == NKC - 1))
                    seg = (0 if istext else 2) + h
                    nc.vector.tensor_add(v[0:P, tc * VD:(tc + 1) * VD],
                                         ps[0:P, 0:VD],
                                         bvt[0:P, seg * VD:(seg + 1) * VD])
                # transposed scores + masked softmax
                eT = etp.tile([128, 4 * 512], bf16, tag="eT")
                psS = psml.tile([128, 512], f32, tag="sml")
                for kc in range(nkt):
                    P, g0 = tiles[kc], starts[kc]
                    ps = pmm.tile([128, 512], f32, tag="mm")
                    for mf in range(3):
                        nc.tensor.matmul(
                            ps[0:P, 0:Ne],
                            r(kT[:, mf * Ne + g0: mf * Ne + g0 + P]),
                            r(qT[:, mf * Ne: mf * Ne + Ne]),
                            start=(mf == 0), stop=(mf == 2))
                    nc.vector.tensor_add(ps[0:P, g0:g0 + P],
                                         ps[0:P, g0:g0 + P],
                                         negI[0:P, 0:P])
                    nc.scalar.activation(
                        eT[0:P, kc * Ne:(kc + 1) * Ne],
                        ps[0:P, 0:Ne], AF.Exp,
                        bias=kmT[0:P, KOFF[j] + kc: KOFF[j] + kc + 1])
                    nc.tensor.matmul(psS[0:1, 0:Ne], ones_bf[0:P, 0:1],
                                     eT[0:P, kc * Ne:(kc + 1) * Ne],
                                     start=(kc == 0), stop=(kc == nkt - 1))
                srow = sinvp.tile([1, 512], f32r, tag="rb")
                nc.vector.tensor_copy(srow[0:1, 0:Ne], psS[0:1, 0:Ne])
                psR = psml.tile([128, 512], f32, tag="sml")
                nc.tensor.matmul(psR[:, 0:Ne], r(ones1[0:1, 0:128]),
                                 srow[0:1, 0:Ne], start=True, stop=True)
                rb = sinvp.tile([128, 512], f32, tag="rb")
                nc.vector.reciprocal_approx_fast(rb[:, 0:Ne], psR[:, 0:Ne])
                for vdc in range(3):
                    psA = pmm.tile([128, 512], f32, tag="mm")
                    for kc in range(nkt):
                        P = tiles[kc]
                        nc.tensor.matmul(
                            psA[:, 0:Ne],
                            v[0:P, kc * VD + vdc * 128: kc * VD + vdc * 128 + 128],
                            eT[0:P, kc * Ne:(kc + 1) * Ne],
                            start=(kc == 0), stop=(kc == nkt - 1))
                    nc.vector.tensor_mul(
                        albumT[:, (h * 3 + vdc) * Ne:(h * 3 + vdc + 1) * Ne],
                        psA[:, 0:Ne], rb[:, 0:Ne])
            return albumT

        def fvta(j, albumT, qfT):
            TB, IB = slots[j]
            Ne = NE[j]
            cq = CQ[j]
            nct = len(cq)
            keysT = keysp.tile([128, NKC * 512], bf16, tag="keysT")
            for dkc in range(NKC):
                ps = pmm.tile([128, 512], f32, tag="mm")
                for vdc in range(NKC):
                    nc.tensor.matmul(
                        ps[:, 0:Ne],
                        wkp_s[:, (vdc * 6 + dkc) * 128:
                              (vdc * 6 + dkc) * 128 + 128],
                        albumT[:, vdc * Ne:(vdc + 1) * Ne],
                        start=(vdc == 0), stop=(vdc == NKC - 1))
                nc.scalar.activation(keysT[:, dkc * Ne:(dkc + 1) * Ne],
                                     ps[:, 0:Ne], AF.Identity,
                                     bias=bkp[:, dkc:dkc + 1])
            # w2 q-major: [q, t]; stationary qfT (24 cols, cheap LDW), the
            # token pad mask rides in as a rank-1 extra contraction term
            psw2 = psml.tile([128, 512], f32, tag="sml")
            for dkc in range(NKC):
                nc.tensor.matmul(
                    psw2[0:LQ, 0:Ne],
                    qfT[:, dkc * LQ:(dkc + 1) * LQ],
                    keysT[:, dkc * Ne:(dkc + 1) * Ne],
                    start=(dkc == 0), stop=False)
            nc.tensor.matmul(psw2[0:LQ, 0:Ne], onesr_f8[0:1, 0:LQ],
                             maskr[0:1, MOFF[j]:MOFF[j] + Ne],
                             start=False, stop=True)
            e2 = smallp.tile([LQ, 512], bf16, tag="e2")
            nc.scalar.activation(e2[0:LQ, 0:Ne], psw2[0:LQ, 0:Ne], AF.Exp)
            s2 = smallp.tile([LQ, 1], f32, tag="s2")
            nc.vector.reduce_sum(s2[0:LQ, :], e2[0:LQ, 0:Ne], axis=AX.X)
            r2 = smallp.tile([LQ, 1], f32, tag="r2")
            nc.vector.reciprocal(r2[0:LQ, :], s2[0:LQ, :])
            wp = smallp.tile([LQ, 1], bf16, tag="wp")
            with nc.allow_low_precision(reason="mean weights; mbar is bf16"):
                nc.vector.tensor_mul(wp[0:LQ, :],
                                     qvalid2[0:LQ, j:j + 1], r2[0:LQ, :])
            pts = ptr.tile([128, 512], f32r, tag="tr")
            for ct in range(nct):
                Q = cq[ct]
                nc.tensor.matmul(
                    p(pts[0:Q, ct:ct + 1]),
                    e2[0:LQ, 128 * ct: 128 * ct + Q],
                    wp[0:LQ, 0:1], start=True, stop=True)
            mbarT = smallp.tile([128, 4], bf16, tag="mbarT")
            nc.vector.tensor_copy(mbarT[:, 0:nct], p(pts[:, 0:nct]))
            vals = valsp.tile([128, 4 * DV], bf16, tag="vals")
            for ct in range(nct):
                Q = cq[ct]
                ps_f0 = pmm.tile([128, 512], f32, tag="mm")
                ps_f1 = pmm.tile([128, 512], f32, tag="mm")
                psf = [ps_f0, ps_f1]
                for vdc in range(NKC):
                    for hf in range(2):
                        nc.tensor.matmul(
                            psf[hf][0:Q, 0:384],
                            albumT[:, vdc * Ne + 128 * ct:
                                   vdc * Ne + 128 * ct + Q],
                            wvp_s[:, (vdc * 2 + hf) * 384:
                                  (vdc * 2 + hf + 1) * 384],
                            start=(vdc == 0), stop=(vdc == NKC - 1))
                for hf in range(2):
                    nc.vector.tensor_copy(
                        vals[0:Q, ct * DV + hf * 384: ct * DV + hf * 384 + 384],
                        psf[hf][0:Q, 0:384])
            for hf in range(2):
                pso = psml.tile([128, 512], f32, tag="sml")
                for ct in range(nct):
                    Q = cq[ct]
                    nc.tensor.matmul(
                        pso[0:1, 0:384], mbarT[0:Q, ct:ct + 1],
                        vals[0:Q, ct * DV + hf * 384: ct * DV + hf * 384 + 384],
                        start=(ct == 0), stop=(ct == nct - 1))
                outrow = smallp.tile([1, 384], f32, tag="e2")
                nc.vector.tensor_copy(outrow[0:1, :], pso[0:1, 0:384])
                nc.sync.dma_start(out_d[j:j + 1, hf * 384:(hf + 1) * 384],
                                  outrow[0:1, :])

        # PE warmup: keep the HAM activity window busy while input/weight
        # DMAs land, so the first real matmuls run at 2.4 GHz
        for _ in range(9):
            ptw = ptr.tile([128, 512], f32r, tag="tr")
            for i in range(4):
                nc.tensor.transpose(r(ptw[:, i * 128:(i + 1) * 128]),
                                    r(ident[:]), r(ident[:]))

        # const v-bias broadcast tiles [128, 4*VD] bf16 (seg = si*2 + h).
        # bv_si borrows the xT pool slot (dead before the first xT write).
        bv_si = xtp.tile([1, 4 * VD], f32r, tag="xT")
        nc.sync.dma_start(bv_si[:], bv_si_d[:])
        bvt = cpool.tile([128, 4 * VD], bf16, tag="bvt")
        for seg in range(4):
            psv = psml.tile([128, 512], f32, tag="sml")
            nc.tensor.matmul(psv[:, 0:VD], r(ones1[0:1, 0:128]),
                             r(bv_si[0:1, seg * VD:(seg + 1) * VD]),
                             start=True, stop=True)
            nc.vector.tensor_copy(bvt[:, seg * VD:(seg + 1) * VD],
                                  psv[:, 0:VD])

        state = preamble(0)

        # resident weights emitted after slot-0 preamble so its input DMAs
        # aren't queued behind ~17MB of weights; h-major, q/k first
        for h in range(H):
            for name in ["wsq", "wsk", "wiq", "wik", "wsv", "wiv"]:
                t = wpool.tile([128, NKC * 384], f32r, tag=f"{name}{h}")
                nc.sync.dma_start(t[:], w_d[name][h])
                wt[(name, h)] = t
        wkp_s = wpool.tile([128, 36 * 128], bf16, tag="wkp")
        nc.sync.dma_start(wkp_s[:], wkp_d[:])
        wvp_s = wpool.tile([128, 12 * 384], bf16, tag="wvp")
        nc.sync.dma_start(wvp_s[:], wvp_d[:])

        for j in range(BL):
            qfT, xT = state
            albumT = compute_heads(j, state)
            if j + 1 < BL:
                state = preamble(j + 1)
            fvta(j, albumT, qfT)

    nc.compile()
    _COMPILED[slots] = nc
    return nc


def _r32(v):
    return int(-(-int(v) // 32) * 32)


def plan_slots(text_lengths, image_lengths):
    """Assign 64 elems to 8 slots x 8 cores minimizing sum over slots of
    (r32(max tl) + r32(max il)). Returns (order[64], slots[8]):
    core c slot j processes elem order[8*j + c]."""
    tl = np.asarray(text_lengths).astype(np.int64)
    il = np.asarray(image_lengths).astype(np.int64)
    idx = np.argsort(-tl, kind="stable")
    assign = []
    for g in range(4):
        grp = idx[g * 16:(g + 1) * 16]
        grp = grp[np.argsort(-il[grp], kind="stable")]
        assign.append(list(grp[:8]))
        assign.append(list(grp[8:]))

    def slot_cost(s):
        return _r32(tl[s].max()) + _r32(il[s].max())

    costs = [slot_cost(s) for s in assign]
    rng = np.random.RandomState(12345)
    for _ in range(30000):
        a = int(rng.randint(8))
        b = int(rng.randint(8))
        if a == b:
            continue
        i = int(rng.randint(8))
        k = int(rng.randint(8))
        sa, sb = assign[a], assign[b]
        sa[i], sb[k] = sb[k], sa[i]
        na, nb = slot_cost(sa), slot_cost(sb)
        if na + nb <= costs[a] + costs[b]:
            costs[a], costs[b] = na, nb
        else:
            sa[i], sb[k] = sb[k], sa[i]
    # big slots first so the kernel tail is the smallest slot
    perm = sorted(range(8), key=lambda s: -costs[s])
    assign = [assign[s] for s in perm]
    slots = tuple((_r32(tl[s].max()), _r32(il[s].max())) for s in assign)
    order = np.array([assign[j][c] for j in range(8) for c in range(8)],
                     dtype=np.int64)
    return order, slots


def make_in_maps(text, images, query, ln_gamma, ln_beta,
                 Wsq, bsq, Wiq, biq, Wsk, bsk, Wik, bik, Wsv, bsv, Wiv, biv,
                 Wkp, bkp, Wvp, bvp,
                 text_lengths, image_lengths, query_lengths):
    """Host-side preprocessing + slot-assigned batch sharding."""
    _ensure_path()
    import ml_dtypes
    f = np.float32
    g = np.asarray(ln_gamma, f)
    beta = np.asarray(ln_beta, f)

    order, slots = plan_slots(text_lengths, image_lengths)
    TP, KT, KS, NE, CQ, KOFF, COFF = _geom(slots)
    TOTK = KOFF[-1]
    TOTC = COFF[-1]

    def fold_w(W):
        return np.asarray(W, f) * g[None, :, None]

    def pack_w(W):
        M = W.shape[2]
        return np.ascontiguousarray(
            W.reshape(H, NKC, 128, M).transpose(0, 2, 1, 3).reshape(H, 128, NKC * M))

    def beta_bias(W, bias):
        Wf = fold_w(W)
        return (np.einsum("d,hdm->hm", beta, Wf) + np.asarray(bias, f)).astype(f)

    ws = {}
    for name, W in [("wsq", Wsq), ("wiq", Wiq), ("wsk", Wsk), ("wik", Wik),
                    ("wsv", Wsv), ("wiv", Wiv)]:
        ws[name] = pack_w(fold_w(W))
    bq_s = beta_bias(Wsq, bsq)
    bk_s = beta_bias(Wsk, bsk)
    bv_s = beta_bias(Wsv, bsv)
    bq_i = beta_bias(Wiq, biq)
    bk_i = beta_bias(Wik, bik)
    bv_i = beta_bias(Wiv, biv)

    # bqk [128, 24]: col=(qk*2+h)*3+mf text, +12 img
    bqk = np.zeros((128, 24), f)
    for qk, (bt, bi) in enumerate([(bq_s, bq_i), (bk_s, bk_i)]):
        for h in range(H):
            for mf in range(3):
                col = (qk * 2 + h) * 3 + mf
                bqk[:, col] = bt[h, mf * 128:(mf + 1) * 128]
                bqk[:, 12 + col] = bi[h, mf * 128:(mf + 1) * 128]
    bv_si = np.concatenate([bv_s[0], bv_s[1], bv_i[0], bv_i[1]]).astype(f)
    bv_si = bv_si.reshape(1, 4 * VD)

    Wkp_ = np.asarray(Wkp, f)
    # [p, (vdc*6+dkc)*128+m] = Wkp[vdc*128+p, dkc*128+m]
    wkp_p = np.ascontiguousarray(
        Wkp_.reshape(NKC, 128, NKC, 128).transpose(1, 0, 2, 3).reshape(128, 36 * 128))
    Wvp_ = np.asarray(Wvp, f)
    wvp_p = np.ascontiguousarray(
        Wvp_.reshape(NKC, 128, 2, 384).transpose(1, 0, 2, 3).reshape(128, 12 * 384))
    bkp_t = np.ascontiguousarray(np.asarray(bkp, f).reshape(6, 128).T)

    tl = np.asarray(text_lengths)
    il = np.asarray(image_lengths)
    ql = np.asarray(query_lengths)

    def rnd(a):
        a = np.ascontiguousarray(np.asarray(a, f))
        return (a.view(np.uint32) & np.uint32(0xFFFFF000)).view(np.float32)

    ident_r = rnd(np.eye(128, dtype=f))
    ones_r = rnd(np.ones((1, 128), f))
    text = rnd(np.asarray(text, f))
    images = rnd(np.asarray(images, f))
    query = rnd(np.asarray(query, f))
    for n in list(ws):
        ws[n] = rnd(ws[n])
    bv_si = rnd(bv_si)
    wkp_b = wkp_p.astype(ml_dtypes.bfloat16)
    wvp_b = wvp_p.astype(ml_dtypes.bfloat16)

    MOFF = [0]
    for ne in NE:
        MOFF.append(MOFF[-1] + ne)

    in_maps = []
    for c in range(NCORES):
        el = [int(order[8 * jj + c]) for jj in range(BL)]
        kmT = np.zeros((128, TOTK), f)
        maskr = np.zeros((1, MOFF[-1]), f)  # cast to bf16 below
        qvalid2 = np.zeros((LQ, BL), f)
        for jj in range(BL):
            e = el[jj]
            TB, IB = slots[jj]
            # key-tile masks: text tiles then img tile
            for t, (P, g0) in enumerate(zip(KT[jj], KS[jj])):
                col = np.zeros(128, f)
                if t < len(TP[jj]):
                    valid = np.arange(128) + 128 * t < tl[e]
                else:
                    valid = np.arange(128) < il[e]
                col[~valid] = MASK
                kmT[:, KOFF[jj] + t] = col
            # FVTA token mask row over the concat axis
            gidx = np.arange(NE[jj])
            valid = np.where(gidx < TB, gidx < tl[e], (gidx - TB) < il[e])
            row = np.zeros(NE[jj], f)
            row[~valid] = -240.0
            maskr[0, MOFF[jj]:MOFF[jj + 1]] = row
            qvalid2[:, jj] = (np.arange(LQ) < ql[e]).astype(f) / LQ
        in_maps.append({
            "text": np.ascontiguousarray(text[el]),
            "images": np.ascontiguousarray(images[el]),
            "query": np.ascontiguousarray(query[el]).astype(ml_dtypes.bfloat16),
            **{n: ws[n] for n in ws},
            "wkp": wkp_b, "wvp": wvp_b,
            "bqk": bqk, "bkp_t": bkp_t, "bv_si": bv_si,
            "kmT": kmT, "maskr": maskr.astype(ml_dtypes.float8_e4m3),
            "qvalid2": qvalid2,
            "ident_r": ident_r, "ones_r": ones_r,
        })
    return in_maps, order, slots


def run(in_maps, slots, trace=False, tmpdir=None):
    _ensure_path()
    from concourse import bass_utils
    nc = build_nc(slots)
    kw = {}
    if trace:
        kw = dict(trace=True, tmpdir=tmpdir)
    res = bass_utils.run_bass_kernel_spmd(nc, in_maps,
                                          core_ids=list(range(NCORES)), **kw)
    return res


def kernel(**inputs):
    in_maps, order, slots = make_in_maps(**inputs)
    res = run(in_maps, slots)
    ql = np.asarray(inputs["query_lengths"]).astype(np.float32)
    bvp_row = np.asarray(inputs["bvp"], np.float32)
    out = np.zeros((B, DV), np.float32)
    for c in range(NCORES):
        for jj in range(BL):
            e = int(order[8 * jj + c])
            out[e] = res.results[c]["out"][jj] + (ql[e] / LQ) * bvp_row
    return out.astype(np.float32)
